# revision 3
# baseline (speedup 1.0000x reference)
"""Expert-parallel top-1 MoE (SwiGLU experts + shared expert) on 8 TRN2 NeuronCores.

Strategy (hardcoded for B=1, T=256, C=1024, H=2048, E=8):
  - Core e holds expert e's weights (host pre-transposed, bf16) plus a
    1/8 slice (along H) of the shared expert.
  - Every core computes router logits in fp32 (exact argmax), builds a
    token->slot permutation matrix for its own expert on-device, gathers
    its tokens with a matmul, runs the SwiGLU FFN on <=128 token slots in
    bf16 (fp32 accumulation), and scatters results back with a matmul,
    accumulating its shared-expert partial in the same PSUM banks.
  - Each core writes a disjoint-support partial of the full [C, T] output;
    the host sums the 8 partials and transposes back to [1, T, C].
"""

import sys

if "/opt/trn_rl_repo" not in sys.path:
    sys.path.insert(0, "/opt/trn_rl_repo")

import ml_dtypes
import numpy as np

B, T, C, H, E = 1, 256, 1024, 2048, 8
HS = H // 8        # shared-expert hidden slice per core
CCAP = 128         # per-expert token capacity (binomial mean 32; 128 is >12 sigma)
BF16 = ml_dtypes.bfloat16

_CACHE = {}


def _build_program():
    import concourse.tile as tile
    from concourse import bacc, mybir

    f32 = mybir.dt.float32
    bf16 = mybir.dt.bfloat16
    u32 = mybir.dt.uint32
    ALU = mybir.AluOpType
    ACT = mybir.ActivationFunctionType

    nc = bacc.Bacc("TRN2", target_bir_lowering=False, debug=False, num_devices=8)

    def din(name, shape, dt):
        return nc.dram_tensor(name, shape, dt, kind="ExternalInput").ap()

    xT32 = din("xT32", [C, T], f32)          # x^T fp32 (router path)
    xb = din("xb", [T, C], bf16)             # x bf16 (gather lhsT)
    xTb = din("xTb", [C, T], bf16)           # x^T bf16 (shared-expert rhs)
    routerT = din("routerT", [C, E], f32)
    upT = din("upT", [C, H], bf16)           # up[e]^T
    gateT = din("gateT", [C, H], bf16)       # gate[e]^T
    downT = din("downT", [H, C], bf16)       # down[e]^T
    wupT = din("wupT", [C, HS], bf16)        # shared up slice^T
    wgateT = din("wgateT", [C, HS], bf16)
    wdownT = din("wdownT", [HS, C], bf16)    # shared down slice^T
    eid = din("eid", [128, 1], f32)          # this core's expert id
    iotaf = din("iotaf", [128, CCAP], f32)   # row-constant 0..CCAP-1 along free
    triu = din("triu", [T, T], bf16)         # triu[i,j] = 1 if i <= j
    idb = din("idb", [128, 128], bf16)       # identity (bf16 transposes)
    outT = nc.dram_tensor("outT", [C, T], f32, kind="ExternalOutput").ap()

    # DRAM views: partition-tiled
    xT32v = xT32.rearrange("(a p) t -> p a t", p=128)      # [128, 8, 256]
    xbv = xb.rearrange("(a p) c -> p a c", p=128)          # [128, 2, 1024]
    xTbv = xTb.rearrange("(a p) t -> p a t", p=128)        # [128, 8, 256]
    routerTv = routerT.rearrange("(a p) e -> p a e", p=128)  # [128, 8, 8]
    upTv = upT.rearrange("(a p) h -> p a h", p=128)        # [128, 8, 2048]
    gateTv = gateT.rearrange("(a p) h -> p a h", p=128)
    downTv = downT.rearrange("(a p) c -> p a c", p=128)    # [128, 16, 1024]
    wupTv = wupT.rearrange("(a p) h -> p a h", p=128)      # [128, 8, 256]
    wgateTv = wgateT.rearrange("(a p) h -> p a h", p=128)
    wdownTv = wdownT.rearrange("(a p) c -> p a c", p=128)  # [128, 2, 1024]
    triuv = triu.rearrange("(a p) t -> p a t", p=128)      # [128, 2, 256]
    outTv = outT.rearrange("(a p) t -> p a t", p=128)      # [128, 8, 256]

    with tile.TileContext(nc) as tc:
        with (
            tc.tile_pool(name="consts", bufs=1) as consts,
            tc.tile_pool(name="wts", bufs=1) as wts,
            tc.tile_pool(name="tmp", bufs=2) as tmp,
        ):
            # ---- input DMAs (small/critical first) ----
            xT32_sb = consts.tile([128, 8, T], f32, tag="xT32")
            nc.sync.dma_start(xT32_sb[:], xT32v[:])
            routerT_sb = consts.tile([128, 8, E], f32, tag="routerT")
            nc.sync.dma_start(routerT_sb[:], routerTv[:])
            eid_sb = consts.tile([128, 1], f32, tag="eid")
            nc.sync.dma_start(eid_sb[:], eid[:])
            iotaf_sb = consts.tile([128, CCAP], f32, tag="iotaf")
            nc.sync.dma_start(iotaf_sb[:], iotaf[:])
            triu_sb = consts.tile([128, 2, T], bf16, tag="triu")
            nc.sync.dma_start(triu_sb[:], triuv[:])
            idb_sb = consts.tile([128, 128], bf16, tag="idb")
            nc.sync.dma_start(idb_sb[:], idb[:])
            xb_sb = consts.tile([128, 2, C], bf16, tag="xb")
            nc.sync.dma_start(xb_sb[:], xbv[:])
            xTb_sb = consts.tile([128, 8, T], bf16, tag="xTb")
            nc.sync.dma_start(xTb_sb[:], xTbv[:])
            wupT_sb = consts.tile([128, 8, HS], bf16, tag="wupT")
            nc.sync.dma_start(wupT_sb[:], wupTv[:])
            wgateT_sb = consts.tile([128, 8, HS], bf16, tag="wgateT")
            nc.sync.dma_start(wgateT_sb[:], wgateTv[:])
            wdownT_sb = consts.tile([128, 2, C], bf16, tag="wdownT")
            nc.sync.dma_start(wdownT_sb[:], wdownTv[:])

            # ---- big expert-weight DMAs, one per K-tile chunk ----
            upk = []
            gatek = []
            for k in range(8):
                t_u = wts.tile([128, H], bf16, tag=f"upk{k}")
                nc.sync.dma_start(t_u[:], upTv[:, k, :])
                upk.append(t_u)
                t_g = wts.tile([128, H], bf16, tag=f"gatek{k}")
                nc.sync.dma_start(t_g[:], gateTv[:, k, :])
                gatek.append(t_g)
            downk = []
            for j in range(16):
                t_d = wts.tile([128, C], bf16, tag=f"downk{j}")
                nc.sync.dma_start(t_d[:], downTv[:, j, :])
                downk.append(t_d)

            # ---- routing: fp32 logits, argmax, mask ----
            mask_sb = consts.tile([128, 2, 1], f32, tag="mask")      # 1.0 if token -> this expert
            maskb_sb = consts.tile([128, 2, 1], bf16, tag="maskb")
            with tc.tile_pool(name="psr", bufs=2, space="PSUM") as psr:
                for tt in range(2):
                    lg_ps = psr.tile([128, E], f32, tag="lg")
                    for k in range(8):
                        nc.tensor.matmul(
                            lg_ps[:],
                            lhsT=xT32_sb[:, k, tt * 128:(tt + 1) * 128],
                            rhs=routerT_sb[:, k, :],
                            start=(k == 0),
                            stop=(k == 7),
                        )
                    lg_sb = tmp.tile([128, E], f32, tag="lg_sb")
                    nc.vector.tensor_copy(lg_sb[:], lg_ps[:])
                    mx = tmp.tile([128, 8], f32, tag="mx")
                    nc.vector.max(mx[:], lg_sb[:])
                    mi = tmp.tile([128, 8], u32, tag="mi")
                    nc.vector.max_index(mi[:], mx[:], lg_sb[:])
                    idxf = tmp.tile([128, 1], f32, tag="idxf")
                    nc.vector.tensor_copy(idxf[:], mi[:, 0:1])
                    nc.vector.tensor_tensor(
                        mask_sb[:, tt, :], idxf[:], eid_sb[:], op=ALU.is_equal
                    )
                    nc.vector.tensor_copy(maskb_sb[:, tt, :], mask_sb[:, tt, :])

                # ---- positions: pos0 = (inclusive prefix count) - 1 via triu matmul ----
                possel_sb = consts.tile([128, 2, 1], f32, tag="possel")
                for mt in range(2):
                    pos_ps = psr.tile([128, 1], f32, tag="pos")
                    for kt in range(2):
                        nc.tensor.matmul(
                            pos_ps[:],
                            lhsT=triu_sb[:, kt, mt * 128:(mt + 1) * 128],
                            rhs=maskb_sb[:, kt, :],
                            start=(kt == 0),
                            stop=(kt == 1),
                        )
                    # possel = (cumsum + 1) * mask - 2:
                    #   routed  -> pos0 = cumsum - 1 in [0, CCAP)
                    #   unrouted -> -2 (never matches a slot)
                    pos1 = tmp.tile([128, 1], f32, tag="pos1")
                    nc.vector.tensor_scalar(
                        pos1[:], pos_ps[:], 1.0, None, op0=ALU.add
                    )
                    posm = tmp.tile([128, 1], f32, tag="posm")
                    nc.vector.tensor_tensor(
                        posm[:], pos1[:], mask_sb[:, mt, :], op=ALU.mult
                    )
                    nc.vector.tensor_scalar(
                        possel_sb[:, mt, :], posm[:], 2.0, None, op0=ALU.subtract
                    )

                # ---- permutation matrices ----
                # permT[t, p] = 1 if token t goes to slot p   [128, 2, CCAP] bf16
                permT_sb = consts.tile([128, 2, CCAP], bf16, tag="permT")
                for tt in range(2):
                    nc.vector.tensor_scalar(
                        permT_sb[:, tt, :],
                        iotaf_sb[:],
                        possel_sb[:, tt, :],
                        None,
                        op0=ALU.is_equal,
                    )
                # perm[p, t] (scatter rhs) via PE transpose
                perm_sb = consts.tile([128, 2 * 128], bf16, tag="perm")
                for tt in range(2):
                    pt_ps = psr.tile([128, 128], bf16, tag="pt")
                    nc.tensor.transpose(pt_ps[:], permT_sb[:, tt, :], idb_sb[:])
                    nc.vector.tensor_copy(
                        perm_sb[:, tt * 128:(tt + 1) * 128], pt_ps[:]
                    )

                # ---- gather: gxT[c, p] = sum_t x[t, c] * permT[t, p] ----
                gx_sb = consts.tile([128, 8, CCAP], bf16, tag="gx")
                for m in range(8):
                    g_ps = psr.tile([128, CCAP], f32, tag="gps")
                    for tt in range(2):
                        nc.tensor.matmul(
                            g_ps[:],
                            lhsT=xb_sb[:, tt, m * 128:(m + 1) * 128],
                            rhs=permT_sb[:, tt, :],
                            start=(tt == 0),
                            stop=(tt == 1),
                        )
                    nc.vector.tensor_copy(gx_sb[:, m, :], g_ps[:])

            # ---- routed-expert FFN (tokens stationary, weights streaming) ----
            hT_sb = consts.tile([128, 16, CCAP], bf16, tag="hT")
            y_sb = consts.tile([128, C], bf16, tag="y")
            with (
                tc.tile_pool(name="psu", bufs=1, space="PSUM") as psu,
                tc.tile_pool(name="psy", bufs=1, space="PSUM") as psy,
                tc.tile_pool(name="pst", bufs=2, space="PSUM") as pst,
            ):
                y_ps = psy.tile([128, C], f32, tag="yps")
                for hh in range(2):
                    u_ps = psu.tile([128, 1024], f32, tag="u")
                    g_ps = psu.tile([128, 1024], f32, tag="g")
                    for k in range(8):
                        for cc in range(2):
                            sl = slice(hh * 1024 + cc * 512, hh * 1024 + (cc + 1) * 512)
                            dst = slice(cc * 512, (cc + 1) * 512)
                            nc.tensor.matmul(
                                u_ps[:, dst], lhsT=gx_sb[:, k, :], rhs=upk[k][:, sl],
                                start=(k == 0), stop=(k == 7),
                            )
                            nc.tensor.matmul(
                                g_ps[:, dst], lhsT=gx_sb[:, k, :], rhs=gatek[k][:, sl],
                                start=(k == 0), stop=(k == 7),
                            )
                    sil_sb = tmp.tile([128, 1024], bf16, tag="sil")
                    nc.scalar.activation(sil_sb[:], g_ps[:], ACT.Silu)
                    h_sb = tmp.tile([128, 1024], bf16, tag="h")
                    nc.vector.tensor_tensor(h_sb[:], sil_sb[:], u_ps[:], op=ALU.mult)
                    for j in range(8):
                        t_ps = pst.tile([128, 128], bf16, tag="tr")
                        nc.tensor.transpose(
                            t_ps[:], h_sb[:, j * 128:(j + 1) * 128], idb_sb[:]
                        )
                        nc.vector.tensor_copy(hT_sb[:, hh * 8 + j, :], t_ps[:])
                    for j in range(8):
                        jj = hh * 8 + j
                        for cc in range(2):
                            dst = slice(cc * 512, (cc + 1) * 512)
                            nc.tensor.matmul(
                                y_ps[:, dst], lhsT=hT_sb[:, jj, :],
                                rhs=downk[jj][:, dst],
                                start=(jj == 0), stop=(jj == 15),
                            )
                nc.vector.tensor_copy(y_sb[:], y_ps[:])

            # ---- shared expert (H/8 slice): hsT[hs, t] ----
            hsT_sb = consts.tile([128, 2, T], bf16, tag="hsT")
            with tc.tile_pool(name="pss", bufs=1, space="PSUM") as pss:
                for st in range(2):
                    us_ps = pss.tile([128, T], f32, tag="us")
                    gs_ps = pss.tile([128, T], f32, tag="gs")
                    for k in range(8):
                        nc.tensor.matmul(
                            us_ps[:], lhsT=wupT_sb[:, k, st * 128:(st + 1) * 128],
                            rhs=xTb_sb[:, k, :], start=(k == 0), stop=(k == 7),
                        )
                        nc.tensor.matmul(
                            gs_ps[:], lhsT=wgateT_sb[:, k, st * 128:(st + 1) * 128],
                            rhs=xTb_sb[:, k, :], start=(k == 0), stop=(k == 7),
                        )
                    sils = tmp.tile([128, T], bf16, tag="sils")
                    nc.scalar.activation(sils[:], gs_ps[:], ACT.Silu)
                    nc.vector.tensor_tensor(
                        hsT_sb[:, st, :], sils[:], us_ps[:], op=ALU.mult
                    )

            # ---- output: scatter routed y + shared-expert down, fused in PSUM ----
            with tc.tile_pool(name="pso", bufs=2, space="PSUM") as pso:
                for m in range(8):
                    o_ps = pso.tile([128, T], f32, tag="o")
                    nc.tensor.matmul(
                        o_ps[:], lhsT=y_sb[:, m * 128:(m + 1) * 128],
                        rhs=perm_sb[:], start=True, stop=False,
                    )
                    for st in range(2):
                        nc.tensor.matmul(
                            o_ps[:], lhsT=wdownT_sb[:, st, m * 128:(m + 1) * 128],
                            rhs=hsT_sb[:, st, :], start=False, stop=(st == 1),
                        )
                    o_sb = tmp.tile([128, T], f32, tag="o_sb")
                    nc.vector.tensor_copy(o_sb[:], o_ps[:])
                    nc.sync.dma_start(outTv[:, m, :], o_sb[:])

    nc.compile()
    return nc


def _get_program():
    if "nc" not in _CACHE:
        _CACHE["nc"] = _build_program()
    return _CACHE["nc"]


def _make_in_maps(x, up, gate, down, router, w_up_s, w_gate_s, w_down_s):
    f32 = np.float32
    x2 = np.ascontiguousarray(x.reshape(T, C)).astype(f32, copy=False)
    xT = np.ascontiguousarray(x2.T)
    common = {
        "xT32": xT,
        "xb": np.ascontiguousarray(x2.astype(BF16)),
        "xTb": np.ascontiguousarray(xT.astype(BF16)),
        "routerT": np.ascontiguousarray(router.astype(f32, copy=False).T),
        "iotaf": np.broadcast_to(
            np.arange(CCAP, dtype=f32)[None, :], (128, CCAP)
        ).copy(),
        "triu": (np.triu(np.ones((T, T), f32))).astype(BF16),
        "idb": np.eye(128, dtype=f32).astype(BF16),
    }
    in_maps = []
    for e in range(E):
        sl = slice(e * HS, (e + 1) * HS)
        m = dict(common)
        m["upT"] = np.ascontiguousarray(up[e].astype(f32, copy=False).T.astype(BF16))
        m["gateT"] = np.ascontiguousarray(gate[e].astype(f32, copy=False).T.astype(BF16))
        m["downT"] = np.ascontiguousarray(down[e].astype(f32, copy=False).T.astype(BF16))
        m["wupT"] = np.ascontiguousarray(w_up_s[sl, :].astype(f32, copy=False).T.astype(BF16))
        m["wgateT"] = np.ascontiguousarray(w_gate_s[sl, :].astype(f32, copy=False).T.astype(BF16))
        m["wdownT"] = np.ascontiguousarray(w_down_s[:, sl].astype(f32, copy=False).T.astype(BF16))
        m["eid"] = np.full((128, 1), float(e), f32)
        in_maps.append(m)
    return in_maps


def run_spmd(in_maps, **kwargs):
    from concourse.bass_utils import run_bass_kernel_spmd

    nc = _get_program()
    return run_bass_kernel_spmd(nc, in_maps, core_ids=list(range(8)), **kwargs)


def kernel(x, up, gate, down, router, w_up_s, w_gate_s, w_down_s):
    in_maps = _make_in_maps(
        np.asarray(x), np.asarray(up), np.asarray(gate), np.asarray(down),
        np.asarray(router), np.asarray(w_up_s), np.asarray(w_gate_s),
        np.asarray(w_down_s),
    )
    res = run_spmd(in_maps)
    acc = np.zeros((C, T), np.float32)
    for i in range(E):
        acc += res.results[i]["outT"]
    return np.ascontiguousarray(acc.T).reshape(B, T, C).astype(np.float32)


# revision 8
# speedup vs baseline: 1.0267x; 1.0267x over previous
"""Expert-parallel top-1 MoE (SwiGLU experts + shared expert) on 8 TRN2 NeuronCores.

Strategy (hardcoded for B=1, T=256, C=1024, H=2048, E=8):
  - Core e holds expert e's weights (host pre-transposed, bf16) plus a
    1/8 slice (along H) of the shared expert.
  - Every core computes router logits in fp32 (exact argmax), builds a
    token->slot permutation matrix for its own expert on-device, gathers
    its tokens with a matmul, runs the SwiGLU FFN on <=128 token slots in
    bf16 (fp32 accumulation), and scatters results back with a matmul,
    accumulating its shared-expert partial in the same PSUM banks.
  - Each core writes a disjoint-support partial of the full [C, T] output;
    the host sums the 8 partials and transposes back to [1, T, C].
"""

import sys

if "/opt/trn_rl_repo" not in sys.path:
    sys.path.insert(0, "/opt/trn_rl_repo")

import ml_dtypes
import numpy as np

B, T, C, H, E = 1, 256, 1024, 2048, 8
HS = H // 8        # shared-expert hidden slice per core
CCAP = 128         # per-expert token capacity (binomial mean 32; 128 is >12 sigma)
BF16 = ml_dtypes.bfloat16

_CACHE = {}


def _build_program():
    import concourse.tile as tile
    from concourse import bacc, mybir

    f32 = mybir.dt.float32
    bf16 = mybir.dt.bfloat16
    u32 = mybir.dt.uint32
    ALU = mybir.AluOpType
    ACT = mybir.ActivationFunctionType

    nc = bacc.Bacc("TRN2", target_bir_lowering=False, debug=False, num_devices=8)

    def din(name, shape, dt):
        return nc.dram_tensor(name, shape, dt, kind="ExternalInput").ap()

    xT32 = din("xT32", [C, T], f32)          # x^T fp32 (router path)
    xb = din("xb", [T, C], bf16)             # x bf16 (gather lhsT)
    xTb = din("xTb", [C, T], bf16)           # x^T bf16 (shared-expert rhs)
    routerT = din("routerT", [C, E], f32)
    upT = din("upT", [C, H], bf16)           # up[e]^T
    gateT = din("gateT", [C, H], bf16)       # gate[e]^T
    downT = din("downT", [H, C], bf16)       # down[e]^T
    wupT = din("wupT", [C, HS], bf16)        # shared up slice^T
    wgateT = din("wgateT", [C, HS], bf16)
    wdownT = din("wdownT", [HS, C], bf16)    # shared down slice^T
    eid = din("eid", [128, 1], f32)          # this core's expert id
    iotaf = din("iotaf", [128, CCAP], f32)   # row-constant 0..CCAP-1 along free
    triu = din("triu", [T, T], bf16)         # triu[i,j] = 1 if i <= j
    idb = din("idb", [128, 128], bf16)       # identity (bf16 transposes)
    outT = nc.dram_tensor("outT", [C, T], f32, kind="ExternalOutput").ap()

    # DRAM views: partition-tiled
    xT32v = xT32.rearrange("(a p) t -> p a t", p=128)      # [128, 8, 256]
    xbv = xb.rearrange("(a p) c -> p a c", p=128)          # [128, 2, 1024]
    xTbv = xTb.rearrange("(a p) t -> p a t", p=128)        # [128, 8, 256]
    routerTv = routerT.rearrange("(a p) e -> p a e", p=128)  # [128, 8, 8]
    upTv = upT.rearrange("(a p) h -> p a h", p=128)        # [128, 8, 2048]
    gateTv = gateT.rearrange("(a p) h -> p a h", p=128)
    downTv = downT.rearrange("(a p) c -> p a c", p=128)    # [128, 16, 1024]
    wupTv = wupT.rearrange("(a p) h -> p a h", p=128)      # [128, 8, 256]
    wgateTv = wgateT.rearrange("(a p) h -> p a h", p=128)
    wdownTv = wdownT.rearrange("(a p) c -> p a c", p=128)  # [128, 2, 1024]
    triuv = triu.rearrange("(a p) t -> p a t", p=128)      # [128, 2, 256]
    outTv = outT.rearrange("(a p) t -> p a t", p=128)      # [128, 8, 256]

    with tile.TileContext(nc) as tc:
        with (
            tc.tile_pool(name="consts", bufs=1) as consts,
            tc.tile_pool(name="wts", bufs=1) as wts,
            tc.tile_pool(name="tmp", bufs=2) as tmp,
        ):
            # ---- critical small DMA first (router path), HWDGE ring 0 ----
            xT32_sb = consts.tile([128, 8, T], f32, tag="xT32")
            nc.sync.dma_start(xT32_sb[:], xT32v[:])

            # ---- big expert-weight DMAs: ~1MB chunks, alternate HWDGE rings ----
            # up/gate: 4 chunks x 2 K-tiles; down: 4 chunks x 4 K-tiles.
            upq, gateq, downq = [], [], []
            for q in range(4):
                t_u = wts.tile([128, 2, H], bf16, tag=f"upq{q}")
                nc.sync.dma_start(t_u[:], upTv[:, 2 * q:2 * q + 2, :])
                upq.append(t_u)
                t_g = wts.tile([128, 2, H], bf16, tag=f"gateq{q}")
                nc.scalar.dma_start(t_g[:], gateTv[:, 2 * q:2 * q + 2, :])
                gateq.append(t_g)
            for q in range(4):
                t_d = wts.tile([128, 4, C], bf16, tag=f"downq{q}")
                eng = nc.sync if q % 2 == 0 else nc.scalar
                eng.dma_start(t_d[:], downTv[:, 4 * q:4 * q + 4, :])
                downq.append(t_d)

            # ---- remaining small inputs on the SWDGE (gpsimd) ring ----
            routerT_sb = consts.tile([128, 8, E], f32, tag="routerT")
            nc.gpsimd.dma_start(routerT_sb[:], routerTv[:])
            xTb_sb = consts.tile([128, 8, T], bf16, tag="xTb")
            nc.gpsimd.dma_start(xTb_sb[:], xTbv[:])
            wupT_sb = consts.tile([128, 8, HS], bf16, tag="wupT")
            nc.gpsimd.dma_start(wupT_sb[:], wupTv[:])
            wgateT_sb = consts.tile([128, 8, HS], bf16, tag="wgateT")
            nc.gpsimd.dma_start(wgateT_sb[:], wgateTv[:])
            triu_sb = consts.tile([128, 2, T], bf16, tag="triu")
            nc.gpsimd.dma_start(triu_sb[:], triuv[:])
            eid_sb = consts.tile([128, 1], f32, tag="eid")
            nc.gpsimd.dma_start(eid_sb[:], eid[:])
            iotaf_sb = consts.tile([128, CCAP], f32, tag="iotaf")
            nc.gpsimd.dma_start(iotaf_sb[:], iotaf[:])
            idb_sb = consts.tile([128, 128], bf16, tag="idb")
            nc.gpsimd.dma_start(idb_sb[:], idb[:])
            xb_sb = consts.tile([128, 2, C], bf16, tag="xb")
            nc.gpsimd.dma_start(xb_sb[:], xbv[:])
            wdownT_sb = consts.tile([128, 2, C], bf16, tag="wdownT")
            nc.gpsimd.dma_start(wdownT_sb[:], wdownTv[:])

            # ---- routing: fp32 logits, argmax, mask; shared expert fills PE ----
            mask_sb = consts.tile([128, 2, 1], f32, tag="mask")      # 1.0 if token -> this expert
            maskb_sb = consts.tile([128, 2, 1], bf16, tag="maskb")
            hsT_sb = consts.tile([128, 2, T], bf16, tag="hsT")
            with tc.tile_pool(name="psA", bufs=2, space="PSUM") as psA:
                for tt in range(2):
                    lg_ps = psA.tile([128, E], f32, tag="lg")
                    for k in range(8):
                        nc.tensor.matmul(
                            lg_ps[:],
                            lhsT=xT32_sb[:, k, tt * 128:(tt + 1) * 128],
                            rhs=routerT_sb[:, k, :],
                            start=(k == 0),
                            stop=(k == 7),
                        )
                    lg_sb = tmp.tile([128, E], f32, tag="lg_sb")
                    nc.vector.tensor_copy(lg_sb[:], lg_ps[:])
                    mx = tmp.tile([128, 8], f32, tag="mx")
                    nc.vector.max(mx[:], lg_sb[:])
                    mi = tmp.tile([128, 8], u32, tag="mi")
                    nc.vector.max_index(mi[:], mx[:], lg_sb[:])
                    idxf = tmp.tile([128, 1], f32, tag="idxf")
                    nc.vector.tensor_copy(idxf[:], mi[:, 0:1])
                    nc.vector.tensor_tensor(
                        mask_sb[:, tt, :], idxf[:], eid_sb[:], op=ALU.is_equal
                    )
                    nc.vector.tensor_copy(maskb_sb[:, tt, :], mask_sb[:, tt, :])

                # shared expert (H/8 slice) on PE while DVE does the routing chain
                for st in range(2):
                    us_ps = psA.tile([128, T], f32, tag="us")
                    gs_ps = psA.tile([128, T], f32, tag="gs")
                    for k in range(8):
                        nc.tensor.matmul(
                            us_ps[:], lhsT=wupT_sb[:, k, st * 128:(st + 1) * 128],
                            rhs=xTb_sb[:, k, :], start=(k == 0), stop=(k == 7),
                        )
                        nc.tensor.matmul(
                            gs_ps[:], lhsT=wgateT_sb[:, k, st * 128:(st + 1) * 128],
                            rhs=xTb_sb[:, k, :], start=(k == 0), stop=(k == 7),
                        )
                    sils = tmp.tile([128, T], bf16, tag="sils")
                    nc.scalar.activation(sils[:], gs_ps[:], ACT.Silu)
                    nc.vector.tensor_tensor(
                        hsT_sb[:, st, :], sils[:], us_ps[:], op=ALU.mult
                    )

                # ---- positions: pos0 = (inclusive prefix count) - 1 via triu matmul ----
                possel_sb = consts.tile([128, 2, 1], f32, tag="possel")
                for mt in range(2):
                    pos_ps = psA.tile([128, 1], f32, tag="lg")
                    for kt in range(2):
                        nc.tensor.matmul(
                            pos_ps[:],
                            lhsT=triu_sb[:, kt, mt * 128:(mt + 1) * 128],
                            rhs=maskb_sb[:, kt, :],
                            start=(kt == 0),
                            stop=(kt == 1),
                        )
                    # possel = (cumsum + 1) * mask - 2:
                    #   routed  -> pos0 = cumsum - 1 in [0, CCAP)
                    #   unrouted -> -2 (never matches a slot)
                    pos1 = tmp.tile([128, 1], f32, tag="pos1")
                    nc.vector.tensor_scalar(
                        pos1[:], pos_ps[:], 1.0, None, op0=ALU.add
                    )
                    posm = tmp.tile([128, 1], f32, tag="posm")
                    nc.vector.tensor_tensor(
                        posm[:], pos1[:], mask_sb[:, mt, :], op=ALU.mult
                    )
                    nc.vector.tensor_scalar(
                        possel_sb[:, mt, :], posm[:], 2.0, None, op0=ALU.subtract
                    )

            # ---- permutation matrices + gather ----
            permT_sb = consts.tile([128, 2, CCAP], bf16, tag="permT")
            perm_sb = consts.tile([128, 2 * 128], bf16, tag="perm")
            gx_sb = consts.tile([128, 8, CCAP], bf16, tag="gx")
            with tc.tile_pool(name="psB", bufs=2, space="PSUM") as psB:
                for tt in range(2):
                    nc.vector.tensor_scalar(
                        permT_sb[:, tt, :],
                        iotaf_sb[:],
                        possel_sb[:, tt, :],
                        None,
                        op0=ALU.is_equal,
                    )
                # perm[p, t] (scatter rhs) via PE transpose
                for tt in range(2):
                    pt_ps = psB.tile([128, 128], bf16, tag="pt")
                    nc.tensor.transpose(pt_ps[:], permT_sb[:, tt, :], idb_sb[:])
                    nc.vector.tensor_copy(
                        perm_sb[:, tt * 128:(tt + 1) * 128], pt_ps[:]
                    )
                # gather: gxT[c, p] = sum_t x[t, c] * permT[t, p]
                for m in range(8):
                    g_ps = psB.tile([128, CCAP], f32, tag="gps")
                    for tt in range(2):
                        nc.tensor.matmul(
                            g_ps[:],
                            lhsT=xb_sb[:, tt, m * 128:(m + 1) * 128],
                            rhs=permT_sb[:, tt, :],
                            start=(tt == 0),
                            stop=(tt == 1),
                        )
                    nc.vector.tensor_copy(gx_sb[:, m, :], g_ps[:])

            # ---- routed-expert FFN (tokens stationary, weights streaming) ----
            hT_sb = consts.tile([128, 16, CCAP], bf16, tag="hT")
            y_sb = consts.tile([128, C], bf16, tag="y")
            with (
                tc.tile_pool(name="psu", bufs=1, space="PSUM") as psu,
                tc.tile_pool(name="psy", bufs=1, space="PSUM") as psy,
                tc.tile_pool(name="pst", bufs=2, space="PSUM") as pst,
            ):
                y_ps = psy.tile([128, C], f32, tag="yps")
                for hh in range(2):
                    u_ps = psu.tile([128, 1024], f32, tag="u")
                    g_ps = psu.tile([128, 1024], f32, tag="g")
                    for k in range(8):
                        for cc in range(2):
                            sl = slice(hh * 1024 + cc * 512, hh * 1024 + (cc + 1) * 512)
                            dst = slice(cc * 512, (cc + 1) * 512)
                            nc.tensor.matmul(
                                u_ps[:, dst], lhsT=gx_sb[:, k, :],
                                rhs=upq[k // 2][:, k % 2, sl],
                                start=(k == 0), stop=(k == 7),
                            )
                            nc.tensor.matmul(
                                g_ps[:, dst], lhsT=gx_sb[:, k, :],
                                rhs=gateq[k // 2][:, k % 2, sl],
                                start=(k == 0), stop=(k == 7),
                            )
                    sil_sb = tmp.tile([128, 1024], bf16, tag="sil")
                    nc.scalar.activation(sil_sb[:], g_ps[:], ACT.Silu)
                    h_sb = tmp.tile([128, 1024], bf16, tag="h")
                    nc.vector.tensor_tensor(h_sb[:], sil_sb[:], u_ps[:], op=ALU.mult)
                    for j in range(8):
                        t_ps = pst.tile([128, 128], bf16, tag="tr")
                        nc.tensor.transpose(
                            t_ps[:], h_sb[:, j * 128:(j + 1) * 128], idb_sb[:]
                        )
                        nc.vector.tensor_copy(hT_sb[:, hh * 8 + j, :], t_ps[:])
                    for j in range(8):
                        jj = hh * 8 + j
                        for cc in range(2):
                            dst = slice(cc * 512, (cc + 1) * 512)
                            nc.tensor.matmul(
                                y_ps[:, dst], lhsT=hT_sb[:, jj, :],
                                rhs=downq[jj // 4][:, jj % 4, dst],
                                start=(jj == 0), stop=(jj == 15),
                            )
                nc.vector.tensor_copy(y_sb[:], y_ps[:])

            # ---- output: scatter routed y + shared-expert down, fused in PSUM ----
            with tc.tile_pool(name="pso", bufs=2, space="PSUM") as pso:
                for m in range(8):
                    o_ps = pso.tile([128, T], f32, tag="o")
                    nc.tensor.matmul(
                        o_ps[:], lhsT=y_sb[:, m * 128:(m + 1) * 128],
                        rhs=perm_sb[:], start=True, stop=False,
                    )
                    for st in range(2):
                        nc.tensor.matmul(
                            o_ps[:], lhsT=wdownT_sb[:, st, m * 128:(m + 1) * 128],
                            rhs=hsT_sb[:, st, :], start=False, stop=(st == 1),
                        )
                    o_sb = tmp.tile([128, T], f32, tag="o_sb")
                    nc.vector.tensor_copy(o_sb[:], o_ps[:])
                    nc.sync.dma_start(outTv[:, m, :], o_sb[:])

    nc.compile()
    return nc


def _get_program():
    if "nc" not in _CACHE:
        _CACHE["nc"] = _build_program()
    return _CACHE["nc"]


def _make_in_maps(x, up, gate, down, router, w_up_s, w_gate_s, w_down_s):
    f32 = np.float32
    x2 = np.ascontiguousarray(x.reshape(T, C)).astype(f32, copy=False)
    xT = np.ascontiguousarray(x2.T)
    common = {
        "xT32": xT,
        "xb": np.ascontiguousarray(x2.astype(BF16)),
        "xTb": np.ascontiguousarray(xT.astype(BF16)),
        "routerT": np.ascontiguousarray(router.astype(f32, copy=False).T),
        "iotaf": np.broadcast_to(
            np.arange(CCAP, dtype=f32)[None, :], (128, CCAP)
        ).copy(),
        "triu": (np.triu(np.ones((T, T), f32))).astype(BF16),
        "idb": np.eye(128, dtype=f32).astype(BF16),
    }
    in_maps = []
    for e in range(E):
        sl = slice(e * HS, (e + 1) * HS)
        m = dict(common)
        m["upT"] = np.ascontiguousarray(up[e].astype(f32, copy=False).T.astype(BF16))
        m["gateT"] = np.ascontiguousarray(gate[e].astype(f32, copy=False).T.astype(BF16))
        m["downT"] = np.ascontiguousarray(down[e].astype(f32, copy=False).T.astype(BF16))
        m["wupT"] = np.ascontiguousarray(w_up_s[sl, :].astype(f32, copy=False).T.astype(BF16))
        m["wgateT"] = np.ascontiguousarray(w_gate_s[sl, :].astype(f32, copy=False).T.astype(BF16))
        m["wdownT"] = np.ascontiguousarray(w_down_s[:, sl].astype(f32, copy=False).T.astype(BF16))
        m["eid"] = np.full((128, 1), float(e), f32)
        in_maps.append(m)
    return in_maps


def run_spmd(in_maps, **kwargs):
    from concourse.bass_utils import run_bass_kernel_spmd

    nc = _get_program()
    return run_bass_kernel_spmd(nc, in_maps, core_ids=list(range(8)), **kwargs)


def kernel(x, up, gate, down, router, w_up_s, w_gate_s, w_down_s):
    in_maps = _make_in_maps(
        np.asarray(x), np.asarray(up), np.asarray(gate), np.asarray(down),
        np.asarray(router), np.asarray(w_up_s), np.asarray(w_gate_s),
        np.asarray(w_down_s),
    )
    res = run_spmd(in_maps)
    acc = np.zeros((C, T), np.float32)
    for i in range(E):
        acc += res.results[i]["outT"]
    return np.ascontiguousarray(acc.T).reshape(B, T, C).astype(np.float32)


# revision 10
# speedup vs baseline: 1.1224x; 1.0932x over previous
"""Expert-parallel top-1 MoE (SwiGLU experts + shared expert) on 8 TRN2 NeuronCores.

Strategy (hardcoded for B=1, T=256, C=1024, H=2048, E=8):
  - Core e holds expert e's weights (host pre-transposed, bf16) plus a
    1/8 slice (along H) of the shared expert.
  - Every core computes router logits in fp32 (exact argmax), builds a
    token->slot permutation matrix for its own expert on-device, gathers
    its tokens with a matmul, runs the SwiGLU FFN on <=128 token slots in
    bf16 (fp32 accumulation), and scatters results back with a matmul,
    accumulating its shared-expert partial in the same PSUM banks.
  - Each core writes a disjoint-support partial of the full [C, T] output;
    the host sums the 8 partials and transposes back to [1, T, C].

Schedule notes:
  - All small inputs ship as two packed buffers (one fp32, one bf16) so the
    routing path lands in ~2 DMAs instead of ~10.
  - Expert weights stream as ~1MB chunks, alternating the two HWDGE rings
    (sync + scalar), ordered by FFN consumption (H-half 0 first, down last).
  - A short burst of dummy matmuls warms the PE clock (HAM) while DMA runs.
"""

import sys

if "/opt/trn_rl_repo" not in sys.path:
    sys.path.insert(0, "/opt/trn_rl_repo")

import ml_dtypes
import numpy as np

B, T, C, H, E = 1, 256, 1024, 2048, 8
HS = H // 8        # shared-expert hidden slice per core
CCAP = 128         # per-expert token capacity (binomial mean 32; 128 is >12 sigma)
BF16 = ml_dtypes.bfloat16

# f32 pack layout (per-partition free offsets)
O_XT32, O_ROUT, O_IOTA, O_EID = 0, 2048, 2112, 2240
F32LEN = 2241
# bf16 pack layout
O_XB, O_XTB, O_WUP, O_WGATE, O_WDOWN, O_TRIU, O_IDB = (
    0, 2048, 4096, 6144, 8192, 10240, 10752)
BFLEN = 10880

N_WARM = 35

_CACHE = {}


def _build_program():
    import concourse.tile as tile
    from concourse import bacc, mybir

    f32 = mybir.dt.float32
    bf16 = mybir.dt.bfloat16
    u32 = mybir.dt.uint32
    ALU = mybir.AluOpType
    ACT = mybir.ActivationFunctionType

    nc = bacc.Bacc("TRN2", target_bir_lowering=False, debug=False, num_devices=8)

    f32pack = nc.dram_tensor("f32pack", [128, F32LEN], f32, kind="ExternalInput").ap()
    bfpack = nc.dram_tensor("bfpack", [128, BFLEN], bf16, kind="ExternalInput").ap()
    upT = nc.dram_tensor("upT", [C, H], bf16, kind="ExternalInput").ap()
    gateT = nc.dram_tensor("gateT", [C, H], bf16, kind="ExternalInput").ap()
    downT = nc.dram_tensor("downT", [H, C], bf16, kind="ExternalInput").ap()
    outT = nc.dram_tensor("outT", [C, T], f32, kind="ExternalOutput").ap()

    upTv = upT.rearrange("(a p) h -> p a h", p=128)        # [128, 8, 2048]
    gateTv = gateT.rearrange("(a p) h -> p a h", p=128)
    downTv = downT.rearrange("(a p) c -> p a c", p=128)    # [128, 16, 1024]
    outTv = outT.rearrange("(a p) t -> p a t", p=128)      # [128, 8, 256]

    with tile.TileContext(nc) as tc:
        with (
            tc.tile_pool(name="consts", bufs=1) as consts,
            tc.tile_pool(name="wts", bufs=1) as wts,
            tc.tile_pool(name="tmp", bufs=2) as tmp,
        ):
            # ---- packed small inputs: one DMA per ring ----
            fp_sb = consts.tile([128, F32LEN], f32, tag="fp")
            nc.sync.dma_start(fp_sb[:], f32pack[:])
            bp_sb = consts.tile([128, BFLEN], bf16, tag="bp")
            nc.scalar.dma_start(bp_sb[:], bfpack[:])

            # slice helpers into the packs
            def xT32s(k, tt):           # fp32 x^T tile [128, 128] (lhsT for logits)
                o = O_XT32 + k * 256 + tt * 128
                return fp_sb[:, o:o + 128]

            def routs(k):               # routerT [128, 8]
                o = O_ROUT + k * 8
                return fp_sb[:, o:o + 8]

            iota_s = fp_sb[:, O_IOTA:O_IOTA + CCAP]
            eid_s = fp_sb[:, O_EID:O_EID + 1]

            def xbs(tt, m):             # x bf16 [128(t), 128(c)]
                o = O_XB + tt * 1024 + m * 128
                return bp_sb[:, o:o + 128]

            def xTbs(k):                # x^T bf16 [128, 256]
                o = O_XTB + k * 256
                return bp_sb[:, o:o + 256]

            def wups(k, st):
                o = O_WUP + k * 256 + st * 128
                return bp_sb[:, o:o + 128]

            def wgates(k, st):
                o = O_WGATE + k * 256 + st * 128
                return bp_sb[:, o:o + 128]

            def wdowns(st, m):
                o = O_WDOWN + st * 1024 + m * 128
                return bp_sb[:, o:o + 128]

            def trius(kt, mt):
                o = O_TRIU + kt * 256 + mt * 128
                return bp_sb[:, o:o + 128]

            idb_s = bp_sb[:, O_IDB:O_IDB + 128]

            # ---- expert weight chunks, ~1MB, consumption order ----
            # up/gate: [128, 4 K-tiles, H-half]; down: [128, 4 K-tiles, C]
            upc = [wts.tile([128, 4, 1024], bf16, tag=f"upc{i}", name=f"upc{i}")
                   for i in range(4)]
            gatec = [wts.tile([128, 4, 1024], bf16, tag=f"gac{i}", name=f"gac{i}")
                     for i in range(4)]
            downc = [wts.tile([128, 4, C], bf16, tag=f"doc{i}", name=f"doc{i}")
                     for i in range(4)]
            for i in range(4):
                hh, kg = i // 2, i % 2
                hsl = slice(hh * 1024, (hh + 1) * 1024)
                nc.sync.dma_start(upc[i][:], upTv[:, kg * 4:(kg + 1) * 4, hsl])
                nc.scalar.dma_start(gatec[i][:], gateTv[:, kg * 4:(kg + 1) * 4, hsl])
            for q in range(4):
                eng = nc.sync if q < 2 else nc.scalar
                eng.dma_start(downc[q][:], downTv[:, q * 4:(q + 1) * 4, :])

            # ---- PE warmup: dummy matmuls while DMA streams ----
            warm_sb = consts.tile([128, 256], bf16, tag="warm")
            nc.vector.memset(warm_sb[:], 0.0)
            with tc.tile_pool(name="psW", bufs=1, space="PSUM") as psW:
                w_ps = psW.tile([128, 128], f32, tag="w")
                for _ in range(N_WARM):
                    nc.tensor.matmul(
                        w_ps[:], lhsT=warm_sb[:, 0:128], rhs=warm_sb[:, 128:256],
                        start=True, stop=True,
                    )

            # ---- routing (fp32 logits) + shared expert on PE ----
            mask_sb = consts.tile([128, 2, 1], f32, tag="mask")
            maskb_sb = consts.tile([128, 2, 1], bf16, tag="maskb")
            hsT_sb = consts.tile([128, 2, T], bf16, tag="hsT")
            possel_sb = consts.tile([128, 2, 1], f32, tag="possel")
            with tc.tile_pool(name="psA", bufs=2, space="PSUM") as psA:
                for tt in range(2):
                    lg_ps = psA.tile([128, E], f32, tag="lg")
                    for k in range(8):
                        nc.tensor.matmul(
                            lg_ps[:], lhsT=xT32s(k, tt), rhs=routs(k),
                            start=(k == 0), stop=(k == 7),
                        )
                    lg_sb = tmp.tile([128, E], f32, tag="lg_sb")
                    nc.vector.tensor_copy(lg_sb[:], lg_ps[:])
                    mx = tmp.tile([128, 8], f32, tag="mx")
                    nc.vector.max(mx[:], lg_sb[:])
                    mi = tmp.tile([128, 8], u32, tag="mi")
                    nc.vector.max_index(mi[:], mx[:], lg_sb[:])
                    idxf = tmp.tile([128, 1], f32, tag="idxf")
                    nc.vector.tensor_copy(idxf[:], mi[:, 0:1])
                    nc.vector.tensor_tensor(
                        mask_sb[:, tt, :], idxf[:], eid_s, op=ALU.is_equal
                    )
                    nc.vector.tensor_copy(maskb_sb[:, tt, :], mask_sb[:, tt, :])

                def shared_half(st):
                    us_ps = psA.tile([128, T], f32, tag="us")
                    gs_ps = psA.tile([128, T], f32, tag="gs")
                    for k in range(8):
                        nc.tensor.matmul(
                            us_ps[:], lhsT=wups(k, st), rhs=xTbs(k),
                            start=(k == 0), stop=(k == 7),
                        )
                        nc.tensor.matmul(
                            gs_ps[:], lhsT=wgates(k, st), rhs=xTbs(k),
                            start=(k == 0), stop=(k == 7),
                        )
                    sils = tmp.tile([128, T], bf16, tag="sils")
                    nc.scalar.activation(sils[:], gs_ps[:], ACT.Silu)
                    nc.vector.tensor_tensor(
                        hsT_sb[:, st, :], sils[:], us_ps[:], op=ALU.mult
                    )

                shared_half(0)

                # positions via triu matmul: cumsum(mask)[t] - 1, unrouted -> -2
                for mt in range(2):
                    pos_ps = psA.tile([128, 1], f32, tag="lg")
                    for kt in range(2):
                        nc.tensor.matmul(
                            pos_ps[:], lhsT=trius(kt, mt), rhs=maskb_sb[:, kt, :],
                            start=(kt == 0), stop=(kt == 1),
                        )
                    pos1 = tmp.tile([128, 1], f32, tag="pos1")
                    nc.vector.tensor_scalar(
                        pos1[:], pos_ps[:], 1.0, None, op0=ALU.add
                    )
                    posm = tmp.tile([128, 1], f32, tag="posm")
                    nc.vector.tensor_tensor(
                        posm[:], pos1[:], mask_sb[:, mt, :], op=ALU.mult
                    )
                    nc.vector.tensor_scalar(
                        possel_sb[:, mt, :], posm[:], 2.0, None, op0=ALU.subtract
                    )

                shared_half(1)

            # ---- permutation matrices + gather ----
            permT_sb = consts.tile([128, 2, CCAP], bf16, tag="permT")
            perm_sb = consts.tile([128, 2 * 128], bf16, tag="perm")
            gx_sb = consts.tile([128, 8, CCAP], bf16, tag="gx")
            with tc.tile_pool(name="psB", bufs=2, space="PSUM") as psB:
                for tt in range(2):
                    nc.vector.tensor_scalar(
                        permT_sb[:, tt, :], iota_s, possel_sb[:, tt, :], None,
                        op0=ALU.is_equal,
                    )
                for tt in range(2):
                    pt_ps = psB.tile([128, 128], bf16, tag="pt")
                    nc.tensor.transpose(pt_ps[:], permT_sb[:, tt, :], idb_s)
                    nc.vector.tensor_copy(
                        perm_sb[:, tt * 128:(tt + 1) * 128], pt_ps[:]
                    )
                for m in range(8):
                    g_ps = psB.tile([128, CCAP], f32, tag="gps")
                    for tt in range(2):
                        nc.tensor.matmul(
                            g_ps[:], lhsT=xbs(tt, m), rhs=permT_sb[:, tt, :],
                            start=(tt == 0), stop=(tt == 1),
                        )
                    nc.vector.tensor_copy(gx_sb[:, m, :], g_ps[:])

            # ---- routed FFN: tokens stationary, weights streaming ----
            hT_sb = consts.tile([128, 16, CCAP], bf16, tag="hT")
            y_sb = consts.tile([128, C], bf16, tag="y")
            with tc.tile_pool(name="psy", bufs=1, space="PSUM") as psy:
                y_ps = psy.tile([128, C], f32, tag="yps")
                with (
                    tc.tile_pool(name="psu", bufs=1, space="PSUM") as psu,
                    tc.tile_pool(name="pst", bufs=2, space="PSUM") as pst,
                ):
                    for hh in range(2):
                        u_ps = psu.tile([128, 1024], f32, tag="u")
                        g_ps = psu.tile([128, 1024], f32, tag="g")
                        for cc in range(2):
                            dst = slice(cc * 512, (cc + 1) * 512)
                            for k in range(8):
                                ch = upc[hh * 2 + k // 4]
                                gh = gatec[hh * 2 + k // 4]
                                wsl = slice(cc * 512, (cc + 1) * 512)
                                nc.tensor.matmul(
                                    u_ps[:, dst], lhsT=gx_sb[:, k, :],
                                    rhs=ch[:, k % 4, wsl],
                                    start=(k == 0), stop=(k == 7),
                                )
                                nc.tensor.matmul(
                                    g_ps[:, dst], lhsT=gx_sb[:, k, :],
                                    rhs=gh[:, k % 4, wsl],
                                    start=(k == 0), stop=(k == 7),
                                )
                            sil = tmp.tile([128, 512], bf16, tag="sil")
                            nc.scalar.activation(sil[:], g_ps[:, dst], ACT.Silu)
                            h_sb = tmp.tile([128, 512], bf16, tag="h")
                            nc.vector.tensor_tensor(
                                h_sb[:], sil[:], u_ps[:, dst], op=ALU.mult
                            )
                            for j4 in range(4):
                                t_ps = pst.tile([128, 128], bf16, tag="tr")
                                nc.tensor.transpose(
                                    t_ps[:], h_sb[:, j4 * 128:(j4 + 1) * 128], idb_s
                                )
                                nc.vector.tensor_copy(
                                    hT_sb[:, hh * 8 + cc * 4 + j4, :], t_ps[:]
                                )

                # ---- down (C-half groups) + fused scatter/shared-down/out ----
                with tc.tile_pool(name="pso", bufs=2, space="PSUM") as pso:
                    for ccc in range(2):
                        dst = slice(ccc * 512, (ccc + 1) * 512)
                        for jj in range(16):
                            nc.tensor.matmul(
                                y_ps[:, dst], lhsT=hT_sb[:, jj, :],
                                rhs=downc[jj // 4][:, jj % 4, dst],
                                start=(jj == 0), stop=(jj == 15),
                            )
                        nc.vector.tensor_copy(y_sb[:, dst], y_ps[:, dst])
                        o_sb = tmp.tile([128, 4 * T], f32, tag="o_sb")
                        for mm in range(4):
                            m = ccc * 4 + mm
                            o_ps = pso.tile([128, T], f32, tag="o")
                            nc.tensor.matmul(
                                o_ps[:], lhsT=y_sb[:, m * 128:(m + 1) * 128],
                                rhs=perm_sb[:], start=True, stop=False,
                            )
                            for st in range(2):
                                nc.tensor.matmul(
                                    o_ps[:], lhsT=wdowns(st, m),
                                    rhs=hsT_sb[:, st, :],
                                    start=False, stop=(st == 1),
                                )
                            nc.vector.tensor_copy(
                                o_sb[:, mm * T:(mm + 1) * T], o_ps[:]
                            )
                        nc.sync.dma_start(
                            outTv[:, ccc * 4:(ccc + 1) * 4, :],
                            o_sb[:].rearrange("p (a t) -> p a t", t=T),
                        )

    nc.compile()
    return nc


def _get_program():
    if "nc" not in _CACHE:
        _CACHE["nc"] = _build_program()
    return _CACHE["nc"]


def _pack_inputs(x, up, gate, down, router, w_up_s, w_gate_s, w_down_s):
    f32 = np.float32
    x2 = np.ascontiguousarray(x.reshape(T, C)).astype(f32, copy=False)
    xT = np.ascontiguousarray(x2.T)

    def fold_cols(a):
        # [R, F] with R = n*128 -> [128, n*F] grouping k-tiles along free dim
        n = a.shape[0] // 128
        return a.reshape(n, 128, a.shape[1]).transpose(1, 0, 2).reshape(128, -1)

    fp = np.zeros((128, F32LEN), f32)
    fp[:, O_XT32:O_XT32 + 2048] = fold_cols(xT)
    fp[:, O_ROUT:O_ROUT + 64] = fold_cols(
        np.ascontiguousarray(router.astype(f32, copy=False).T))
    fp[:, O_IOTA:O_IOTA + CCAP] = np.arange(CCAP, dtype=f32)[None, :]

    bp = np.zeros((128, BFLEN), BF16)
    bp[:, O_XB:O_XB + 2048] = fold_cols(x2).astype(BF16)
    bp[:, O_XTB:O_XTB + 2048] = fold_cols(xT).astype(BF16)
    bp[:, O_TRIU:O_TRIU + 512] = fold_cols(np.triu(np.ones((T, T), f32))).astype(BF16)
    bp[:, O_IDB:O_IDB + 128] = np.eye(128, dtype=f32).astype(BF16)

    in_maps = []
    for e in range(E):
        sl = slice(e * HS, (e + 1) * HS)
        fpe = fp.copy()
        fpe[:, O_EID] = float(e)
        bpe = bp.copy()
        bpe[:, O_WUP:O_WUP + 2048] = fold_cols(
            np.ascontiguousarray(w_up_s[sl, :].astype(f32, copy=False).T)).astype(BF16)
        bpe[:, O_WGATE:O_WGATE + 2048] = fold_cols(
            np.ascontiguousarray(w_gate_s[sl, :].astype(f32, copy=False).T)).astype(BF16)
        bpe[:, O_WDOWN:O_WDOWN + 2048] = fold_cols(
            np.ascontiguousarray(w_down_s[:, sl].astype(f32, copy=False).T)).astype(BF16)
        m = {
            "f32pack": fpe,
            "bfpack": bpe,
            "upT": np.ascontiguousarray(up[e].astype(f32, copy=False).T.astype(BF16)),
            "gateT": np.ascontiguousarray(gate[e].astype(f32, copy=False).T.astype(BF16)),
            "downT": np.ascontiguousarray(down[e].astype(f32, copy=False).T.astype(BF16)),
        }
        in_maps.append(m)
    return in_maps


_make_in_maps = _pack_inputs


def run_spmd(in_maps, **kwargs):
    from concourse.bass_utils import run_bass_kernel_spmd

    nc = _get_program()
    return run_bass_kernel_spmd(nc, in_maps, core_ids=list(range(8)), **kwargs)


def kernel(x, up, gate, down, router, w_up_s, w_gate_s, w_down_s):
    in_maps = _pack_inputs(
        np.asarray(x), np.asarray(up), np.asarray(gate), np.asarray(down),
        np.asarray(router), np.asarray(w_up_s), np.asarray(w_gate_s),
        np.asarray(w_down_s),
    )
    res = run_spmd(in_maps)
    acc = np.zeros((C, T), np.float32)
    for i in range(E):
        acc += res.results[i]["outT"]
    return np.ascontiguousarray(acc.T).reshape(B, T, C).astype(np.float32)
